# revision 1
# baseline (speedup 1.0000x reference)
"""Trainium2 Bass kernel for NeatModule forward (gnn_message_passing).

Strategy (8 NeuronCores, SPMD):
  - Full batch (128) everywhere; each topo layer's destination nodes are
    sharded across the 8 cores round-robin (406 nodes per core per layer).
  - Node state lives in HBM as [20000 nodes, 128 batch] fp32.
  - Layers 0 and 1 (small source ranges: 512 / 3760 nodes) are computed as
    dense matmuls from SBUF-resident sources - no gathers at all.
  - For layers 2-5, each core gathers the state rows of its edges' source
    nodes with one indirect DMA per 128-edge tile (per-partition row
    offsets, 512B rows), landing as msgs tiles [128 edges, 128 batch].
  - The weighted segment-sum over edges is a sequence of PE matmuls:
    host-built A tiles [128 edges, 32 dst] (one-hot by destination, scaled
    by the edge weight) are the stationary operand; each matmul accumulates
    into a 32-row window of a PSUM block, using tile_position col-groups.
  - Activations (sigmoid/tanh/relu by act_id) are applied per PSUM block on
    the Scalar engine, combined with mask-predicated copies on Vector.
  - Each core's 406 computed rows are AllGather'ed and written back into
    the HBM state, which unblocks the next layer's gathers.

Host-side prep prunes edges to the output-reachable subgraph, shards and
packs them into the tile/window structure (identical instruction stream on
all cores; per-core data differs, zero-padded to shared tile counts).
"""
import numpy as np

import concourse.bass as bass
import concourse.mybir as mybir
import concourse.tile as tile
from concourse.vector_clock import ScopedClock, VectorClock
from concourse.tile_rust import add_dep_helper
from concourse.bass_utils import run_bass_kernel_spmd

NUM_INPUTS = 512
NUM_OUTPUTS = 256
NUM_NODES = 20000
NUM_LAYERS = 6
CHUNK = (NUM_NODES - NUM_INPUTS) // NUM_LAYERS  # 3248
NCORES = 8
SLICE = CHUNK // NCORES                         # 406
WPL = (SLICE + 31) // 32                        # 13 windows of 32 dst / layer
NBLK = (WPL + 3) // 4                           # 4 PSUM blocks / layer
P = 128
GCH = 32                                        # gather chunk, in 128-edge tiles

# ---------------------------------------------------------------- tile fixes


def _drain_and_barrier(self, tick_clock, wait_clock):
    # This walrus build rejects instructions carrying many sync waits; emit
    # one nop per proc instead of a single drain waiting on everything.
    gc = tick_clock.global_clock
    n = len(gc)
    for p in range(n):
        t = gc[p]
        if t > 0:
            nop = self.nc.sync.nop(nofuse=True)
            vec = [0] * n
            vec[p] = t
            wait_clock.add_sem_waits(nop.ins, ScopedClock({None: VectorClock(vec)}))
    self.nc.sync.drain()
    self.nc.all_engine_barrier()
    assert self.sems is not None
    popped = self.nc._tile_sem_poison_stack.pop()
    assert popped is self._sem_poison
    self.nc.clear_and_free_semaphores(list(self.sems.allocated().values()))
    self.nc.all_engine_barrier()


tile.TileContext._drain_and_barrier = _drain_and_barrier


def split_waits(nc, K=1):
    """Hoist overflow sync waits onto InstNoOps inserted just before, same engine."""
    n_split = 0
    for f in nc.m.functions:
        for bb in f.blocks:
            insts = list(bb.instructions)
            out = []
            changed = False
            for inst in insts:
                si = inst.sync_info
                if si is not None and si.on_wait is not None and len(si.on_wait) > K:
                    waits = list(si.on_wait)
                    over, keep = waits[:-K], waits[-K:]
                    for j in range(0, len(over), K):
                        out.append(mybir.InstNoOp(
                            name=f"I-waitsplit-{nc.next_id()}",
                            engine=inst.engine,
                            sync_info=mybir.SyncInfo(on_wait=over[j:j + K], on_update=[]),
                        ))
                    si.on_wait = keep
                    inst.sync_info = si
                    changed = True
                    n_split += 1
                out.append(inst)
            if changed:
                bb.instructions = out
    return n_split


# ------------------------------------------------------------------ host prep


def _prune_edges(edge_src, edge_dst):
    """Keep only edges that transitively feed the last NUM_OUTPUTS nodes."""
    needed = np.zeros(NUM_NODES, dtype=bool)
    needed[NUM_NODES - NUM_OUTPUTS:] = True
    layer_of = np.full(NUM_NODES, -1)
    for l in range(NUM_LAYERS):
        layer_of[NUM_INPUTS + l * CHUNK: NUM_INPUTS + (l + 1) * CHUNK] = l
    dst_l = layer_of[edge_dst]
    keep = np.zeros(len(edge_src), dtype=bool)
    for l in range(NUM_LAYERS - 1, -1, -1):
        m = (dst_l == l) & needed[edge_dst]
        keep |= m
        needed[edge_src[m]] = True
    return keep


def prep(weights, edge_src, edge_dst, act_id):
    es = np.asarray(edge_src).astype(np.int64)
    ed = np.asarray(edge_dst).astype(np.int64)
    ew = np.asarray(weights).astype(np.float32)
    act_id = np.asarray(act_id).astype(np.int64)

    keep = _prune_edges(es, ed)
    es, ed, ew = es[keep], ed[keep], ew[keep]

    # dst node -> (core, window, col): round-robin over cores for balance
    rel = (ed - NUM_INPUTS) % CHUNK          # position within its layer
    e_l = (ed - NUM_INPUTS) // CHUNK
    e_nc = rel % NCORES                      # owning core (round-robin)
    srel = rel // NCORES                     # slot within core slice
    e_win = srel // 32
    e_col = srel % 32

    # tiles per (layer, window): shared across cores (max). Empty windows get
    # zero tiles; their dst rows are pruned nodes nobody reads.
    # Layers 0 and 1 are handled DENSELY (small source ranges) - no gathers.
    counts = np.zeros((NCORES, NUM_LAYERS, WPL), dtype=np.int64)
    np.add.at(counts, (e_nc, e_l, e_win), 1)
    tiles_lw = -(-counts.max(axis=0) // P)
    tiles_lw[0, :] = 0
    tiles_lw[1, :] = 0
    tiles_lw[2, :] = 0
    tiles_lw[3, :] = 0

    tile_off = np.zeros((NUM_LAYERS, WPL), dtype=np.int64)
    off = 0
    for l in range(NUM_LAYERS):
        for w in range(WPL):
            tile_off[l, w] = off
            off += tiles_lw[l, w]
    T_total = int(off)
    T_layer = tiles_lw.sum(axis=1).astype(int)

    src_idx = np.zeros((NCORES, P, T_total), dtype=np.int32)
    A = np.zeros((NCORES, P, 32 * T_total), dtype=np.float32)

    for i in range(NCORES):
        m = e_nc == i
        s_i, w_i = es[m], ew[m]
        l_i, win_i, col_i = e_l[m], e_win[m], e_col[m]
        order = np.lexsort((col_i, win_i, l_i))
        s_i, w_i, l_i, win_i, col_i = (a[order] for a in (s_i, w_i, l_i, win_i, col_i))
        key = l_i * WPL + win_i
        g0 = np.searchsorted(key, np.arange(NUM_LAYERS * WPL), side="left")
        g1 = np.searchsorted(key, np.arange(NUM_LAYERS * WPL), side="right")
        for l in range(4, NUM_LAYERS):
            for w in range(WPL):
                a0, a1 = g0[l * WPL + w], g1[l * WPL + w]
                if a1 == a0:
                    continue
                slots = np.arange(a1 - a0)
                t = tile_off[l, w] + slots // P
                pp = slots % P
                src_idx[i, pp, t] = s_i[a0:a1]
                A[i, pp, 32 * t + col_i[a0:a1]] = w_i[a0:a1]

    # dense weights for layers 0/1/2 (sources < 512 / 3760 / 7008)
    DCH = [4, 30, 55, 81]              # 128-row source chunks per dense layer
    W0 = np.zeros((NCORES, P, DCH[0] * WPL * 32), dtype=np.float32)
    W1 = np.zeros((NCORES, P, DCH[1] * WPL * 32), dtype=np.float32)
    W2 = np.zeros((NCORES, P, DCH[2] * WPL * 32), dtype=np.float32)
    W3 = np.zeros((NCORES, P, DCH[3] * WPL * 32), dtype=np.float32)
    for l, W in ((0, W0), (1, W1), (2, W2), (3, W3)):
        m = e_l == l
        s, wt, i_, wn, cl = es[m], ew[m], e_nc[m], e_win[m], e_col[m]
        ch, pp = s // P, s % P
        np.add.at(W, (i_, pp, (ch * WPL + wn) * 32 + cl), wt)

    # activation masks per (core, layer, block): mask over the 128 PSUM rows
    msig = np.zeros((NCORES, P, NUM_LAYERS * NBLK), dtype=np.int8)
    mtnh = np.zeros((NCORES, P, NUM_LAYERS * NBLK), dtype=np.int8)
    for i in range(NCORES):
        for l in range(NUM_LAYERS):
            base = NUM_INPUTS + l * CHUNK
            for b in range(NBLK):
                rows = np.arange(128)
                slot = 128 * b + rows            # dst slot within core slice
                valid = slot < SLICE
                node = base + NCORES * slot[valid] + i
                col = l * NBLK + b
                msig[i, rows[valid], col] = (act_id[node] == 0)
                mtnh[i, rows[valid], col] = (act_id[node] == 1)

    meta = dict(T_total=T_total, tiles_lw=tiles_lw, tile_off=tile_off, T_layer=T_layer)
    data = dict(src_idx=src_idx, A=A, msig=msig, mtnh=mtnh, W0=W0, W1=W1, W2=W2, W3=W3)
    return meta, data


# -------------------------------------------------------------- kernel build


def build_nc(meta, reps=1):
    T_total = meta["T_total"]
    tiles_lw = meta["tiles_lw"]
    tile_off = meta["tile_off"]
    T_layer = meta["T_layer"]
    f32 = mybir.dt.float32

    nc = bass.Bass()
    xn = nc.declare_dram_parameter("xn", [NUM_INPUTS, P], f32, isOutput=False)
    sidx = nc.declare_dram_parameter("sidx", [P, T_total], mybir.dt.int32, isOutput=False)
    A_in = nc.declare_dram_parameter("A_in", [P, 32 * T_total], f32, isOutput=False)
    msig_in = nc.declare_dram_parameter("msig", [P, NUM_LAYERS * NBLK], mybir.dt.int8, isOutput=False)
    mtnh_in = nc.declare_dram_parameter("mtnh", [P, NUM_LAYERS * NBLK], mybir.dt.int8, isOutput=False)
    DCH = [4, 30, 55, 81]
    W0_in = nc.declare_dram_parameter("W0", [P, DCH[0] * WPL * 32], f32, isOutput=False)
    W1_in = nc.declare_dram_parameter("W1", [P, DCH[1] * WPL * 32], f32, isOutput=False)
    W2_in = nc.declare_dram_parameter("W2", [P, DCH[2] * WPL * 32], f32, isOutput=False)
    W3_in = nc.declare_dram_parameter("W3", [P, DCH[3] * WPL * 32], f32, isOutput=False)
    out = nc.declare_dram_parameter("out", [NUM_OUTPUTS, P], f32, isOutput=True)

    state = nc.dram_tensor("state", [NUM_NODES, P], f32)
    contrib = nc.dram_tensor("contrib", [SLICE, P], f32)
    gathered = nc.dram_tensor("gathered", [CHUNK, P], f32, addr_space="Shared")

    AF = mybir.ActivationFunctionType

    with tile.TileContext(nc) as tc:
        with (
            tc.tile_pool(name="big", bufs=1) as big,
            tc.tile_pool(name="msgs", bufs=24) as msgsp,
            tc.tile_pool(name="stage", bufs=2) as stage,
            tc.tile_pool(name="ps", bufs=4, space="PSUM") as psp,
        ):
            A_sb = big.tile([P, 32 * T_total], f32, name="A_sb")
            idx_sb = big.tile([P, T_total], mybir.dt.int32, name="idx_sb")
            msig_sb = big.tile([P, NUM_LAYERS * NBLK], mybir.dt.int8, name="msig_sb")
            mtnh_sb = big.tile([P, NUM_LAYERS * NBLK], mybir.dt.int8, name="mtnh_sb")
            W0_sb = big.tile([P, DCH[0] * WPL * 32], f32, name="W0_sb")
            W1_sb = big.tile([P, DCH[1] * WPL * 32], f32, name="W1_sb")
            nc.sync.dma_start(W0_sb[:], W0_in[:])
            nc.sync.dma_start(W1_sb[:], W1_in[:])
            nc.sync.dma_start(A_sb[:], A_in[:])
            nc.sync.dma_start(idx_sb[:], sidx[:])
            nc.sync.dma_start(msig_sb[:], msig_in[:])
            nc.sync.dma_start(mtnh_sb[:], mtnh_in[:])
            x_dma = nc.sync.dma_start(state[:NUM_INPUTS, :], xn[:])
            # zero the dense layer-1 source over-read region (rows 3760..3840):
            # its weights are zero but PE computes 0*garbage (NaN poisoning)
            zt = big.tile([P, P], f32, name="zt")
            nc.vector.memset(zt[:], 0.0)
            pad_lo = NUM_INPUTS + CHUNK          # 3760
            pad_n = 30 * P - pad_lo              # 80
            nc.sync.dma_start(state[pad_lo:pad_lo + pad_n, :], zt[:pad_n, :])
            pad_lo2 = NUM_INPUTS + 2 * CHUNK     # 7008
            pad_n2 = 55 * P - pad_lo2            # 32
            nc.sync.dma_start(state[pad_lo2:pad_lo2 + pad_n2, :], zt[:pad_n2, :])
            pad_lo3 = NUM_INPUTS + 3 * CHUNK     # 10256
            pad_n3 = 81 * P - pad_lo3            # 112
            x_dma = nc.sync.dma_start(state[pad_lo3:pad_lo3 + pad_n3, :], zt[:pad_n3, :])

            prev_sync = x_dma          # instruction whose completion gates next layer's gathers
            prev_coll = None           # last collective (for contrib WAR)
            for r in range(reps):
                for l in range(NUM_LAYERS):
                    dense = l < 4
                    if dense:
                        ndch = DCH[l]
                        W_sb = W0_sb if l == 0 else (W1_sb if l == 1 else None)
                        Wd_in = W2_in if l == 2 else W3_in
                        # copy the source range of state into SBUF once per layer
                        srcs = stage.tile([P, ndch * P], f32, name=f"srcs_{r}_{l}", tag="srcs", bufs=1)
                        sc = nc.sync.dma_start(
                            srcs[:].rearrange("p (c b) -> p c b", b=P),
                            state[:ndch * P, :].rearrange("(c p) b -> p c b", p=P))
                        add_dep_helper(sc.ins, prev_sync.ins, reason="state RAW dense")
                    else:
                        toff = int(tile_off[l, 0])
                        Tl = int(T_layer[l])
                        mts = []
                        for tt in range(Tl):
                            mt = msgsp.tile([P, P], f32, name=f"m_{r}_{l}_{tt}", tag="msgs")
                            g = nc.gpsimd.indirect_dma_start(
                                out=mt[:, :],
                                out_offset=None,
                                in_=state[:, :],
                                in_offset=bass.IndirectOffsetOnAxis(
                                    ap=idx_sb[:, toff + tt: toff + tt + 1], axis=0),
                            )
                            add_dep_helper(g.ins, prev_sync.ins, reason="state RAW")
                            mts.append(mt)

                    dmas = []
                    for b in range(NBLK):
                        ps = psp.tile([P, P], f32, name=f"ps_{r}_{l}_{b}", tag="ps")
                        wlo, whi = 4 * b, min(4 * b + 4, WPL)
                        if dense:
                            nw = whi - wlo
                            M = nw * 32
                            if W_sb is None:
                                # stream this block's W2 columns from HBM
                                wb = msgsp.tile([P, DCH[l] * M], f32,
                                                name=f"wb_{r}_{l}_{b}", tag="wbuf", bufs=2)
                                nc.sync.dma_start(
                                    wb[:].rearrange("p (c q) -> p c q", q=M),
                                    Wd_in[:].rearrange("p (c q) -> p c q", q=WPL * 32)[
                                        :, :, wlo * 32:wlo * 32 + M])
                            # one matmul per source chunk covering the whole
                            # 4-window block (their 32-col groups are contiguous)
                            for c4 in range(DCH[l]):
                                if W_sb is None:
                                    lhs = wb[:, c4 * M:(c4 + 1) * M]
                                else:
                                    lhs = W_sb[:, (c4 * WPL + wlo) * 32:
                                               (c4 * WPL + wlo) * 32 + M]
                                nc.tensor.matmul(
                                    ps[:M, :],
                                    lhsT=lhs,
                                    rhs=srcs[:, c4 * P:(c4 + 1) * P],
                                    start=(c4 == 0), stop=(c4 == DCH[l] - 1),
                                )
                        else:
                            blk_tiles = int(tiles_lw[l, wlo:whi].sum())
                            if blk_tiles == 0:
                                # all-empty block (pruned region): rows must still be
                                # defined in case an unpruned zero-in-degree node lands here
                                nc.vector.memset(ps[:], 0.0)
                            for w in range(wlo, whi):
                                j = w % 4
                                nt = int(tiles_lw[l, w])
                                t0 = int(tile_off[l, w])
                                for k in range(nt):
                                    gt = t0 + k
                                    lt = gt - toff
                                    nc.tensor.matmul(
                                        ps[32 * j:32 * (j + 1), :],
                                        lhsT=A_sb[:, 32 * gt:32 * (gt + 1)],
                                        rhs=mts[lt][:, :],
                                        start=(k == 0), stop=(k == nt - 1),
                                        tile_position=(0, 32 * j),
                                    )
                        res = stage.tile([P, P], f32, name=f"res_{r}_{l}_{b}", tag="res")
                        sig = stage.tile([P, P], f32, name=f"sig_{r}_{l}_{b}", tag="sig")
                        tnh = stage.tile([P, P], f32, name=f"tnh_{r}_{l}_{b}", tag="tnh")
                        col = l * NBLK + b
                        nc.scalar.activation(res[:], ps[:], AF.Relu)
                        nc.scalar.activation(sig[:], ps[:], AF.Sigmoid)
                        nc.scalar.activation(tnh[:], ps[:], AF.Tanh)
                        nc.vector.copy_predicated(
                            res[:], msig_sb[:, col:col + 1].to_broadcast([P, P]), sig[:])
                        nc.vector.copy_predicated(
                            res[:], mtnh_sb[:, col:col + 1].to_broadcast([P, P]), tnh[:])
                        rows = min(P, SLICE - 128 * b)
                        d = nc.sync.dma_start(contrib[128 * b:128 * b + rows, :], res[:rows, :])
                        if prev_coll is not None:
                            add_dep_helper(d.ins, prev_coll.ins, reason="contrib WAR")
                        dmas.append(d)

                    coll = nc.gpsimd.collective_compute(
                        "AllGather", mybir.AluOpType.bypass,
                        replica_groups=[list(range(NCORES))],
                        ins=[contrib[:]], outs=[gathered[:]],
                    )
                    for d in dmas:
                        add_dep_helper(coll.ins, d.ins, reason="contrib RAW")
                    add_dep_helper(coll.ins, prev_sync.ins, reason="gathered WAR vs prev state copy")
                    base = NUM_INPUTS + l * CHUNK
                    # gathered row (i*SLICE + s) holds node rel = NCORES*s + i
                    st = nc.sync.dma_start(
                        state[base:base + CHUNK, :].rearrange("(s i) b -> i s b", i=NCORES),
                        gathered[:].rearrange("(i s) b -> i s b", i=NCORES),
                    )
                    add_dep_helper(st.ins, coll.ins, reason="gathered RAW")
                    prev_sync = st
                    prev_coll = coll

            od = nc.sync.dma_start(out[:], state[NUM_NODES - NUM_OUTPUTS:, :])
            add_dep_helper(od.ins, prev_sync.ins, reason="out RAW")

    split_waits(nc)
    return nc


# ---------------------------------------------------------------- entry point

_CACHE = {}


def _get_compiled(meta, reps=1):
    key = (meta["T_total"], tuple(meta["T_layer"]), reps)
    if key not in _CACHE:
        _CACHE[key] = build_nc(meta, reps=reps)
    return _CACHE[key]


def kernel(x, weights, edge_src, edge_dst, act_id, layer_masks, steps=1, _reps=1):
    x = np.asarray(x, dtype=np.float32)
    meta, data = prep(weights, edge_src, edge_dst, act_id)
    nc = _get_compiled(meta, reps=_reps)
    xn = np.ascontiguousarray(x.T)  # [512 nodes, 128 batch]
    in_maps = [
        {
            "xn": xn,
            "sidx": data["src_idx"][i],
            "A_in": data["A"][i],
            "msig": data["msig"][i],
            "mtnh": data["mtnh"][i],
            "W0": data["W0"][i],
            "W1": data["W1"][i],
            "W2": data["W2"][i],
            "W3": data["W3"][i],
        }
        for i in range(NCORES)
    ]
    res = run_bass_kernel_spmd(nc, in_maps, list(range(NCORES)))
    return np.ascontiguousarray(res.results[0]["out"].T)  # [128, 256]



# revision 3
# speedup vs baseline: 2.5544x; 2.5544x over previous
"""Trainium2 Bass kernel for NeatModule forward (gnn_message_passing).

Strategy (8 NeuronCores, SPMD, fp16 data / f32 PSUM accumulation):
  - Full batch (128) everywhere; each topo layer's destination nodes are
    sharded round-robin across the 8 cores (406 real + 10 pad slots -> 416
    slots per core per layer).
  - Node state lives in HBM as fp16 [17152 rows, 128 batch] in a PERMUTED
    core-major layout: layer l's region is rows [512+l*3328, 512+(l+1)*3328),
    core c's slots at offset c*416. An AllGather of the per-core [416,128]
    contrib then lands each layer's values directly in place - no
    permutation copy.
  - Layers 0-3 are dense matmuls: fp16 weight matrices [128, DCH*416] are
    SBUF-resident (loaded once), and the source states are kept in
    SBUF-resident fp16 tiles (state_in + one per layer region), updated
    from HBM after each layer's collective. Matmuls are emitted chunk-major
    so chunks from older layers execute while the previous layer's
    collective is still in flight.
  - Layers 4-5 gather each edge's source row (256B) with indirect DMA into
    [128 edge, 128 batch] fp16 tiles; host-built one-hot A tiles
    [128 edge, 32 dst] scaled by edge weight are the stationary operand of
    PE matmuls accumulating 32-row windows via tile_position col groups.
  - Activations (sigmoid/tanh/relu by act_id) on Scalar, combined with
    mask-predicated copies on Vector; results cast to fp16 for the
    collective.
  - Layer 5 computes only the 32 output nodes each core owns (edges into
    non-output nodes are pruned): no collective - each core DMAs its
    [32,128] f32 result to its own `out` and the host assembles the full
    [128,256] output from all 8 cores.

Host prep prunes edges to the output-reachable subgraph and packs weights /
A tiles / gather indices in the permuted storage layout (identical
instruction stream on all cores; per-core data differs, zero-padded to
shared tile counts).
"""
import numpy as np

import concourse.bass as bass
import concourse.mybir as mybir
import concourse.tile as tile
from concourse.vector_clock import ScopedClock, VectorClock
from concourse.tile_rust import add_dep_helper
from concourse.bass_utils import run_bass_kernel_spmd

NUM_INPUTS = 512
NUM_OUTPUTS = 256
NUM_NODES = 20000
NUM_LAYERS = 6
CHUNK = (NUM_NODES - NUM_INPUTS) // NUM_LAYERS  # 3248 real nodes per layer
NCORES = 8
SLICE = CHUNK // NCORES                         # 406 real slots per core
CSLICE = 416                                    # padded slots (13 windows of 32)
WPL = CSLICE // 32                              # 13
REGION = NCORES * CSLICE                        # 3328 storage rows per layer
NSTORE = NUM_INPUTS + 5 * REGION                # 17152 (layers 0-4 stored)
P = 128
DCH = [(NUM_INPUTS + l * REGION) // P for l in range(4)]  # [4, 30, 56, 82]
NMCOL = 21                                      # mask cols: 5 layers x 4 blocks + L5

# ---------------------------------------------------------------- tile fixes


def _drain_and_barrier(self, tick_clock, wait_clock):
    # This walrus build rejects instructions carrying many sync waits; emit
    # one nop per proc instead of a single drain waiting on everything.
    gc = tick_clock.global_clock
    n = len(gc)
    for p in range(n):
        t = gc[p]
        if t > 0:
            nop = self.nc.sync.nop(nofuse=True)
            vec = [0] * n
            vec[p] = t
            wait_clock.add_sem_waits(nop.ins, ScopedClock({None: VectorClock(vec)}))
    self.nc.sync.drain()
    self.nc.all_engine_barrier()
    assert self.sems is not None
    popped = self.nc._tile_sem_poison_stack.pop()
    assert popped is self._sem_poison
    self.nc.clear_and_free_semaphores(list(self.sems.allocated().values()))
    self.nc.all_engine_barrier()


tile.TileContext._drain_and_barrier = _drain_and_barrier


def split_waits(nc, K=1):
    """Hoist overflow sync waits onto InstNoOps inserted just before, same engine."""
    n_split = 0
    for f in nc.m.functions:
        for bb in f.blocks:
            insts = list(bb.instructions)
            out = []
            changed = False
            for inst in insts:
                si = inst.sync_info
                if si is not None and si.on_wait is not None and len(si.on_wait) > K:
                    waits = list(si.on_wait)
                    over, keep = waits[:-K], waits[-K:]
                    for j in range(0, len(over), K):
                        out.append(mybir.InstNoOp(
                            name=f"I-waitsplit-{nc.next_id()}",
                            engine=inst.engine,
                            sync_info=mybir.SyncInfo(on_wait=over[j:j + K], on_update=[]),
                        ))
                    si.on_wait = keep
                    inst.sync_info = si
                    changed = True
                    n_split += 1
                out.append(inst)
            if changed:
                bb.instructions = out
    return n_split


# ------------------------------------------------------------------ host prep


def _prune_edges(edge_src, edge_dst):
    """Keep only edges that transitively feed the last NUM_OUTPUTS nodes."""
    needed = np.zeros(NUM_NODES, dtype=bool)
    needed[NUM_NODES - NUM_OUTPUTS:] = True
    layer_of = np.full(NUM_NODES, -1)
    for l in range(NUM_LAYERS):
        layer_of[NUM_INPUTS + l * CHUNK: NUM_INPUTS + (l + 1) * CHUNK] = l
    dst_l = layer_of[edge_dst]
    keep = np.zeros(len(edge_src), dtype=bool)
    for l in range(NUM_LAYERS - 1, -1, -1):
        m = (dst_l == l) & needed[edge_dst]
        keep |= m
        needed[edge_src[m]] = True
    return keep


def _srow(nodes):
    """Storage row of a node (valid for inputs + layers 0-4)."""
    nodes = np.asarray(nodes)
    l = (nodes - NUM_INPUTS) // CHUNK
    rel = (nodes - NUM_INPUTS) % CHUNK
    r = NUM_INPUTS + l * REGION + (rel % NCORES) * CSLICE + rel // NCORES
    return np.where(nodes < NUM_INPUTS, nodes, r).astype(np.int64)


def prep(weights, edge_src, edge_dst, act_id):
    es = np.asarray(edge_src).astype(np.int64)
    ed = np.asarray(edge_dst).astype(np.int64)
    ew = np.asarray(weights).astype(np.float32).astype(np.float16)
    act_id = np.asarray(act_id).astype(np.int64)

    keep = _prune_edges(es, ed)
    es, ed, ew = es[keep], ed[keep], ew[keep]

    s_row = _srow(es)                       # src storage rows (srcs always < L5)
    e_l = (ed - NUM_INPUTS) // CHUNK
    rel_d = (ed - NUM_INPUTS) % CHUNK
    core = (rel_d % NCORES).astype(np.int64)
    slot = (rel_d // NCORES).astype(np.int64)           # dst slot, layers 0-4
    j5 = ed - (NUM_NODES - NUM_OUTPUTS)                 # L5 kept edges: dst is output
    assert np.all(j5[e_l == 5] >= 0)
    core = np.where(e_l == 5, j5 % NCORES, core)
    slot = np.where(e_l == 5, j5 // NCORES, slot)       # L5 slot in [0,32)

    # dense fp16 weights, layers 0-3: [core][128 part, DCH*416]
    W = [np.zeros((NCORES, P, DCH[l] * CSLICE), dtype=np.float16) for l in range(4)]
    for l in range(4):
        m = e_l == l
        ch, pp = s_row[m] // P, s_row[m] % P
        np.add.at(W[l], (core[m], pp, ch * CSLICE + slot[m]), ew[m])

    # sparse tiles, layers 4-5
    win = slot // 32
    cl = slot % 32
    cnt4 = np.zeros((NCORES, WPL), dtype=np.int64)
    m4 = e_l == 4
    np.add.at(cnt4, (core[m4], win[m4]), 1)
    tiles4 = -(-cnt4.max(axis=0) // P)                  # per-window tile count
    off4 = np.concatenate(([0], np.cumsum(tiles4)))[:WPL]
    T4 = int(tiles4.sum())
    cnt5 = np.zeros(NCORES, dtype=np.int64)
    m5 = e_l == 5
    np.add.at(cnt5, core[m5], 1)
    T5 = int(-(-cnt5.max() // P))
    T_total = T4 + T5

    src_idx = np.zeros((NCORES, P, T_total), dtype=np.int32)
    A = np.zeros((NCORES, P, 32 * T_total), dtype=np.float16)
    for i in range(NCORES):
        mi = m4 & (core == i)
        s_i, w_i, wn_i, cl_i = s_row[mi], ew[mi], win[mi], cl[mi]
        order = np.lexsort((cl_i, wn_i))
        s_i, w_i, wn_i, cl_i = (a[order] for a in (s_i, w_i, wn_i, cl_i))
        g0 = np.searchsorted(wn_i, np.arange(WPL), side="left")
        g1 = np.searchsorted(wn_i, np.arange(WPL), side="right")
        for w in range(WPL):
            a0, a1 = g0[w], g1[w]
            if a1 == a0:
                continue
            slots = np.arange(a1 - a0)
            t = off4[w] + slots // P
            pp = slots % P
            src_idx[i, pp, t] = s_i[a0:a1]
            A[i, pp, 32 * t + cl_i[a0:a1]] = w_i[a0:a1]
        mi = m5 & (core == i)
        s_i, w_i, sl_i = s_row[mi], ew[mi], slot[mi]
        slots = np.arange(len(s_i))
        t = T4 + slots // P
        pp = slots % P
        src_idx[i, pp, t] = s_i
        A[i, pp, 32 * t + sl_i] = w_i

    # activation masks per (core, layer, block): over the block's PSUM rows
    msig = np.zeros((NCORES, P, NMCOL), dtype=np.int8)
    mtnh = np.zeros((NCORES, P, NMCOL), dtype=np.int8)
    for i in range(NCORES):
        for l in range(5):
            base = NUM_INPUTS + l * CHUNK
            for b in range(4):
                rows = np.arange(P if b < 3 else 32)
                sl = 128 * b + rows
                valid = sl < SLICE
                node = base + NCORES * sl[valid] + i
                colm = l * 4 + b
                msig[i, rows[valid], colm] = (act_id[node] == 0)
                mtnh[i, rows[valid], colm] = (act_id[node] == 1)
        rows = np.arange(32)
        node = (NUM_NODES - NUM_OUTPUTS) + NCORES * rows + i
        msig[i, rows, 20] = (act_id[node] == 0)
        mtnh[i, rows, 20] = (act_id[node] == 1)

    meta = dict(tiles4=tuple(int(t) for t in tiles4), off4=tuple(int(o) for o in off4),
                T4=T4, T5=T5, T_total=T_total)
    data = dict(src_idx=src_idx, A=A, msig=msig, mtnh=mtnh, W=W)
    return meta, data


def make_in_maps(data, xn):
    return [
        {
            "xn": xn,
            "sidx": data["src_idx"][i],
            "A_in": data["A"][i],
            "msig": data["msig"][i],
            "mtnh": data["mtnh"][i],
            "W0": data["W"][0][i],
            "W1": data["W"][1][i],
            "W2": data["W"][2][i],
            "W3": data["W"][3][i],
        }
        for i in range(NCORES)
    ]


# -------------------------------------------------------------- kernel build


def build_nc(meta, reps=1):
    tiles4 = meta["tiles4"]
    off4 = meta["off4"]
    T4, T5, T_total = meta["T4"], meta["T5"], meta["T_total"]
    f32 = mybir.dt.float32
    f16 = mybir.dt.float16

    nc = bass.Bass()
    xn = nc.declare_dram_parameter("xn", [NUM_INPUTS, P], f16, isOutput=False)
    sidx = nc.declare_dram_parameter("sidx", [P, T_total], mybir.dt.int32, isOutput=False)
    A_in = nc.declare_dram_parameter("A_in", [P, 32 * T_total], f16, isOutput=False)
    msig_in = nc.declare_dram_parameter("msig", [P, NMCOL], mybir.dt.int8, isOutput=False)
    mtnh_in = nc.declare_dram_parameter("mtnh", [P, NMCOL], mybir.dt.int8, isOutput=False)
    W_in = [nc.declare_dram_parameter(f"W{l}", [P, DCH[l] * CSLICE], f16, isOutput=False)
            for l in range(4)]
    out = nc.declare_dram_parameter("out", [32, P], f32, isOutput=True)

    state = nc.dram_tensor("state", [NSTORE, P], f16, addr_space="Shared")
    contribs = [nc.dram_tensor(f"contrib{l}", [CSLICE, P], f16) for l in range(5)]

    AF = mybir.ActivationFunctionType

    with tile.TileContext(nc) as tc:
        with (
            tc.tile_pool(name="big", bufs=1) as big,
            tc.tile_pool(name="msgs", bufs=16) as msgsp,
            tc.tile_pool(name="stage", bufs=3) as stage,
            tc.tile_pool(name="ps", bufs=8, space="PSUM") as psp,
        ):
            W_sb = [big.tile([P, DCH[l] * CSLICE], f16, name=f"W{l}_sb") for l in range(4)]
            A_sb = big.tile([P, 32 * T_total], f16, name="A_sb")
            idx_sb = big.tile([P, T_total], mybir.dt.int32, name="idx_sb")
            msig_sb = big.tile([P, NMCOL], mybir.dt.int8, name="msig_sb")
            mtnh_sb = big.tile([P, NMCOL], mybir.dt.int8, name="mtnh_sb")
            state_in = big.tile([P, 4 * P], f16, name="state_in")
            sl = [big.tile([P, 26 * P], f16, name=f"sl{l}") for l in range(3)]
            for l in range(4):
                nc.sync.dma_start(W_sb[l][:], W_in[l][:])
            nc.sync.dma_start(A_sb[:], A_in[:])
            nc.sync.dma_start(idx_sb[:], sidx[:])
            nc.sync.dma_start(msig_sb[:], msig_in[:])
            nc.sync.dma_start(mtnh_sb[:], mtnh_in[:])
            nc.sync.dma_start(
                state_in[:].rearrange("p (c b) -> p c b", b=P),
                xn[:].rearrange("(c p) b -> p c b", p=P))
            x_dma = nc.sync.dma_start(state[:NUM_INPUTS, :], xn[:])

            prev_od = x_dma            # previous rep's final consumer of state
            prev_coll = [None] * 5     # per-layer collective of previous rep
            for r in range(reps):
                colls = [None] * 5
                for l in range(NUM_LAYERS):
                    ps = [psp.tile([P, P], f32, name=f"ps_{r}_{l}_{b}", tag="ps")
                          for b in range(4 if l < 5 else 1)]
                    if l < 4:
                        # dense: chunk-major so older-layer chunks run during
                        # the previous layer's collective
                        chunks = [state_in[:, c * P:(c + 1) * P] for c in range(4)]
                        for k in range(l):
                            chunks += [sl[k][:, c * P:(c + 1) * P] for c in range(26)]
                        ndch = DCH[l]
                        assert len(chunks) == ndch
                        for c in range(ndch):
                            for b in range(4):
                                M = P if b < 3 else 32
                                nc.tensor.matmul(
                                    ps[b][:M, :],
                                    lhsT=W_sb[l][:, c * CSLICE + b * P:
                                                 c * CSLICE + b * P + M],
                                    rhs=chunks[c],
                                    start=(c == 0), stop=(c == ndch - 1),
                                )
                    else:
                        t0l = 0 if l == 4 else T4
                        Tl = T4 if l == 4 else T5
                        gate = colls[3] if l == 4 else colls[4]
                        mts = []
                        for t in range(Tl):
                            mt = msgsp.tile([P, P], f16, name=f"m_{r}_{l}_{t}", tag="msgs")
                            g = nc.gpsimd.indirect_dma_start(
                                out=mt[:, :],
                                out_offset=None,
                                in_=state[:, :],
                                in_offset=bass.IndirectOffsetOnAxis(
                                    ap=idx_sb[:, t0l + t: t0l + t + 1], axis=0),
                            )
                            add_dep_helper(g.ins, gate.ins, reason="state RAW")
                            mts.append(mt)
                        if l == 4:
                            for w in range(WPL):
                                b, jw = w // 4, w % 4
                                nt = tiles4[w]
                                for k in range(nt):
                                    gt = off4[w] + k
                                    nc.tensor.matmul(
                                        ps[b][32 * jw:32 * (jw + 1), :],
                                        lhsT=A_sb[:, 32 * gt:32 * (gt + 1)],
                                        rhs=mts[gt][:, :],
                                        start=(k == 0), stop=(k == nt - 1),
                                        tile_position=(0, 32 * jw),
                                    )
                        else:
                            for k in range(T5):
                                gt = T4 + k
                                nc.tensor.matmul(
                                    ps[0][0:32, :],
                                    lhsT=A_sb[:, 32 * gt:32 * (gt + 1)],
                                    rhs=mts[k][:, :],
                                    start=(k == 0), stop=(k == T5 - 1),
                                    tile_position=(0, 0),
                                )

                    if l < 5:
                        dmas = []
                        for b in range(4):
                            rows = P if b < 3 else 32
                            colm = l * 4 + b
                            res = stage.tile([P, P], f16, name=f"res_{r}_{l}_{b}", tag="res")
                            sig = stage.tile([P, P], f16, name=f"sig_{r}_{l}_{b}", tag="sig")
                            tnh = stage.tile([P, P], f16, name=f"tnh_{r}_{l}_{b}", tag="tnh")
                            nc.scalar.activation(res[:rows], ps[b][:rows], AF.Relu)
                            nc.scalar.activation(sig[:rows], ps[b][:rows], AF.Sigmoid)
                            nc.scalar.activation(tnh[:rows], ps[b][:rows], AF.Tanh)
                            nc.vector.copy_predicated(
                                res[:rows],
                                msig_sb[:rows, colm:colm + 1].to_broadcast([rows, P]),
                                sig[:rows])
                            nc.vector.copy_predicated(
                                res[:rows],
                                mtnh_sb[:rows, colm:colm + 1].to_broadcast([rows, P]),
                                tnh[:rows])
                            d = nc.sync.dma_start(
                                contribs[l][P * b:P * b + rows, :], res[:rows, :])
                            if prev_coll[l] is not None:
                                add_dep_helper(d.ins, prev_coll[l].ins,
                                               reason="contrib WAR")
                            dmas.append(d)
                        base = NUM_INPUTS + l * REGION
                        coll = nc.gpsimd.collective_compute(
                            "AllGather", mybir.AluOpType.bypass,
                            replica_groups=[list(range(NCORES))],
                            ins=[contribs[l][:]],
                            outs=[state[base:base + REGION, :]],
                        )
                        for d in dmas:
                            add_dep_helper(coll.ins, d.ins, reason="contrib RAW")
                        add_dep_helper(coll.ins, prev_od.ins,
                                       reason="state WAR vs prev rep readers")
                        colls[l] = coll
                        prev_coll[l] = coll
                        if l <= 2:
                            cp = nc.sync.dma_start(
                                sl[l][:].rearrange("p (c b) -> p c b", b=P),
                                state[base:base + REGION, :].rearrange(
                                    "(c p) b -> p c b", p=P))
                            add_dep_helper(cp.ins, coll.ins, reason="sl RAW")
                    else:
                        res = stage.tile([P, P], f32, name=f"res5_{r}", tag="res5", bufs=2)
                        sig = stage.tile([P, P], f32, name=f"sig5_{r}", tag="sig5", bufs=2)
                        tnh = stage.tile([P, P], f32, name=f"tnh5_{r}", tag="tnh5", bufs=2)
                        nc.scalar.activation(res[0:32], ps[0][0:32], AF.Relu)
                        nc.scalar.activation(sig[0:32], ps[0][0:32], AF.Sigmoid)
                        nc.scalar.activation(tnh[0:32], ps[0][0:32], AF.Tanh)
                        nc.vector.copy_predicated(
                            res[0:32],
                            msig_sb[0:32, 20:21].to_broadcast([32, P]), sig[0:32])
                        nc.vector.copy_predicated(
                            res[0:32],
                            mtnh_sb[0:32, 20:21].to_broadcast([32, P]), tnh[0:32])
                        od = nc.sync.dma_start(out[:, :], res[0:32, :])
                        add_dep_helper(od.ins, prev_od.ins, reason="out WAW")
                        prev_od = od

    split_waits(nc)
    return nc


# ---------------------------------------------------------------- entry point

_CACHE = {}


def _get_compiled(meta, reps=1):
    key = (meta["T_total"], meta["tiles4"], meta["T5"], reps)
    if key not in _CACHE:
        _CACHE[key] = build_nc(meta, reps=reps)
    return _CACHE[key]


def kernel(x, weights, edge_src, edge_dst, act_id, layer_masks, steps=1, _reps=1):
    x = np.asarray(x, dtype=np.float32)
    meta, data = prep(weights, edge_src, edge_dst, act_id)
    nc = _get_compiled(meta, reps=_reps)
    xn = np.ascontiguousarray(x.T).astype(np.float16)  # [512 nodes, 128 batch]
    in_maps = make_in_maps(data, xn)
    res = run_bass_kernel_spmd(nc, in_maps, list(range(NCORES)))
    nodes = np.empty((NUM_OUTPUTS, 128), dtype=np.float32)
    for c in range(NCORES):
        nodes[c::NCORES] = res.results[c]["out"]
    return np.ascontiguousarray(nodes.T)  # [128, 256]
